# revision 7
# baseline (speedup 1.0000x reference)
"""CosHead kernel for Trainium2 (8 NeuronCores, data-parallel over batch).

Computes out[b,c,h,w] = 10 * scale[c] * cos_sim(x[b,:,h,w], weights[c,:])
 = (x[b,:,hw] . wn_scaled[c,:]) / ||x[b,:,hw]||
where wn_scaled[c,:] = weights[c,:] / ||weights[c,:]|| * scale[c] * 10.

v4 design (v1 f32: 86.8us, v2: 83.6us, v3: 69.0us):
  - bf16 end-to-end: x bf16 (8.4MB/core read), out bf16 (2.6MB/core
    write), upcast on host -> DMA floor ~31us/core at ~358GB/s share.
  - host-side weight prep uploaded as [128, 240] bf16 [wnT0|wnT1|ones].
  - norm inversion: ONE ACT pass via Abs_reciprocal_sqrt (direct
    InstActivation emission; dedicated TRN2 table, range 40000 >> norm^2
    ~450; measured rel err 4e-3 vs the 2e-2 budget).
  - v3 lesson (HAM log): PE oscillated k=8/8 <-> k=4/8 (27.5us at half
    duty, MM avg 456ns vs 215 warm) because in-order PE waited on
    same-group squares/psum consumers every 1024-col group. v4 runs a
    3-stage software pipeline at 512-col group granularity so every
    engine only touches results >= 1 group old:
      PE:  [gemm(g) x2MM, norm(g-1) x2MM]   (norm reads g-1's x^2)
      ACT: [sq_a(tile), rsqrt(g-2)]
      DVE: [sq_v(tile), mul(g-2)]
      GpSimd: [sq_g(tile), store-trigger(batch of 4 groups)]
    psum rings: pg (gemm) bufs=4, pn (norm) bufs=3 -> 7 of 8 banks; the
    deep pg ring lets PE run 2-3 groups ahead of the DVE muls.
  - squares per 1024-col load-tile, split by measured rates: DVE 832
    cols (bf16 2x_1P ~0.5ns/col), ACT 704 (~0.8ns/col), GpSimd 512
    (bf16 Q7 upconvert makes it slow: 1.76ns/col).
  - loads on sync HWDGE queue; stores batched [80, 2048] on gpsimd
    software-DGE, triggers one pipeline stage late so they never block.
"""

import os
import sys

import numpy as np

for _p in ("/opt/trn_rl_repo",):
    if os.path.isdir(_p) and _p not in sys.path:
        sys.path.append(_p)

B, D, C = 8, 256, 80
HW = 128 * 128
LTILE = 1024   # DMA load tile (hw cols)
G = 512        # pipeline group (hw cols) == one psum bank
P = 128        # SBUF partitions / d-chunk size
N_CORES = 8
STB = 4        # groups per output store batch

# squares: column split of each [128, 2*LTILE] block across engines,
# balanced against each engine's fixed load (DVE: muls, ACT: rsqrt,
# GpSimd: store triggers) using measured rates.
SQ_V = 1088  # DVE, bf16 2x_1P (~0.5ns/col)
SQ_A = 512   # ACT (~0.85ns/col)
# remainder (2*LTILE - SQ_V - SQ_A = 448) on GpSimd (bf16 1.76ns/col)

_NC_CACHE = {}


def _act_direct(nc, out, in_, func, scale=1.0):
    """Emit InstActivation directly (same lowering as nc.scalar.activation)
    for funcs the wrapper refuses (Abs_reciprocal_sqrt is table-backed on
    TRN2 but gated behind the Rsqrt accuracy warning)."""
    from concourse import mybir

    eng = nc.scalar
    bias = eng.bass.const_aps.scalar_like(0.0, in_)
    ins = [eng.lower_ap(in_), eng.lower_ap(bias)]
    ins.append(mybir.ImmediateValue(dtype=mybir.dt.float32, value=scale))
    ins.append(mybir.ImmediateValue(dtype=mybir.dt.float32, value=0.0))
    return eng.add_instruction(
        mybir.InstActivation(
            name=eng.bass.get_next_instruction_name(),
            func=func,
            ins=ins,
            outs=[eng.lower_ap(out)],
        )
    )


def build_bass_kernel(hw: int = HW, use_rsqrt: bool = True):
    """Build the single-core Bass program (SPMD: all cores run this)."""
    import concourse.bass as bass
    import concourse.tile as tile
    from concourse import bacc, mybir

    f32 = mybir.dt.float32
    bf16 = mybir.dt.bfloat16

    ng = hw // G          # pipeline groups
    gpt = LTILE // G      # groups per load tile

    nc = bacc.Bacc("TRN2", target_bir_lowering=False, debug=False)
    x_d = nc.declare_dram_parameter("x", [D, hw], bf16, isOutput=False)
    w_d = nc.declare_dram_parameter("wpack", [P, 3 * C], bf16, isOutput=False)
    out_d = nc.declare_dram_parameter("out", [C, hw], bf16, isOutput=True)

    with tile.TileContext(nc) as tc:
        with (
            tc.tile_pool(name="setup", bufs=1) as setup,
            tc.tile_pool(name="xp", bufs=5) as xp,
            tc.tile_pool(name="x2p", bufs=4) as x2p,
            tc.tile_pool(name="invp", bufs=3) as invp,
            tc.tile_pool(name="outp", bufs=3) as outp,
            tc.tile_pool(name="pg", bufs=4, space=bass.MemorySpace.PSUM) as pgp,
            tc.tile_pool(name="pn", bufs=3, space=bass.MemorySpace.PSUM) as pnp,
        ):
            wp = setup.tile([P, 3 * C], bf16)
            wnT0 = wp[:, 0:C]
            wnT1 = wp[:, C : 2 * C]
            ones = wp[:, 2 * C : 3 * C]

            # [256,hw] viewed as [128 partitions, 2 d-chunks, hw]
            x_src = x_d[:, :].rearrange("(c p) w -> p c w", c=2)
            wp_loaded = False

            xt = [None] * (hw // LTILE)   # load tiles
            x2t = [None] * (hw // LTILE)  # squared load tiles
            pgs = [None] * ng
            pns = [None] * ng
            invs = [None] * ng
            outs = [None] * (ng // STB)

            def xslice(g, chunk, x_list):
                lt, h = g // gpt, g % gpt
                base = chunk * LTILE + h * G
                return x_list[lt][:, base : base + G]

            for g in range(ng + 2):
                # -- load + squares, once per load tile --
                if g < ng and g % gpt == 0:
                    lt = g // gpt
                    lo = lt * LTILE
                    x_sb = xp.tile([P, 2 * LTILE], bf16)
                    nc.sync.dma_start(
                        out=x_sb[:].rearrange("p (c w) -> p c w", c=2),
                        in_=x_src[:, :, lo : lo + LTILE],
                    )
                    xt[lt] = x_sb
                    if not wp_loaded:
                        # weights load AFTER the first x tile: x is on the
                        # critical path, wpack only gates the first matmul
                        nc.sync.dma_start(out=wp, in_=w_d[:, :])
                        wp_loaded = True
                    x2_sb = x2p.tile([P, 2 * LTILE], bf16)
                    nc.vector.tensor_mul(
                        x2_sb[:, :SQ_V], x_sb[:, :SQ_V], x_sb[:, :SQ_V]
                    )
                    if SQ_A:
                        nc.scalar.square(
                            x2_sb[:, SQ_V : SQ_V + SQ_A],
                            x_sb[:, SQ_V : SQ_V + SQ_A],
                        )
                    nc.gpsimd.tensor_mul(
                        x2_sb[:, SQ_V + SQ_A :],
                        x_sb[:, SQ_V + SQ_A :],
                        x_sb[:, SQ_V + SQ_A :],
                    )
                    x2t[lt] = x2_sb

                # -- PE: norm(g-1) first (its deps are a stage older,
                # so the in-order PE is least likely to stall), then gemm(g)
                if 0 <= g - 1 < ng:
                    pn = pnp.tile([C, G], f32, tag="pn")
                    nc.tensor.matmul(
                        pn, ones, xslice(g - 1, 0, x2t), start=True, stop=False
                    )
                    nc.tensor.matmul(
                        pn, ones, xslice(g - 1, 1, x2t), start=False, stop=True
                    )
                    pns[g - 1] = pn
                if g < ng:
                    pg = pgp.tile([C, G], f32, tag="pg")
                    nc.tensor.matmul(
                        pg, wnT0, xslice(g, 0, xt), start=True, stop=False
                    )
                    nc.tensor.matmul(
                        pg, wnT1, xslice(g, 1, xt), start=False, stop=True
                    )
                    pgs[g] = pg

                # -- ACT: rsqrt(g-2); DVE: mul(g-2); GpSimd: store batch --
                k = g - 2
                if 0 <= k < ng:
                    inv = invp.tile([C, G], f32, tag="inv")
                    if use_rsqrt:
                        _act_direct(
                            nc, inv, pns[k],
                            mybir.ActivationFunctionType.Abs_reciprocal_sqrt,
                        )
                    else:
                        sq = invp.tile([C, G], f32, tag="sq")
                        nc.scalar.sqrt(sq, pns[k])
                        nc.vector.reciprocal_approx_fast(inv, sq)
                    invs[k] = inv
                    pns[k] = None

                    if k % STB == 0:
                        outs[k // STB] = outp.tile(
                            [C, STB * G], bf16, name=f"ob{k // STB}"
                        )
                    o_sb = outs[k // STB]
                    nc.vector.tensor_mul(
                        o_sb[:, (k % STB) * G : (k % STB + 1) * G],
                        pgs[k],
                        invs[k],
                    )
                    pgs[k] = None
                    invs[k] = None
                    if k % STB == STB - 1:
                        sb = k // STB
                        # last two batches drain on the scalar HWDGE queue:
                        # ACT is idle by then and hw-DGE drains faster than
                        # gpsimd software-DGE, shortening the tail
                        q = nc.scalar if sb >= (ng // STB) - 2 else nc.gpsimd
                        q.dma_start(
                            out=out_d[:, sb * STB * G : (sb + 1) * STB * G],
                            in_=o_sb,
                        )

    nc.compile()
    return nc


def prepare_wpack(weights, adaptive_scale_factor):
    """Host-side weight prep: [128, 240] bf16 = [wnT0 | wnT1 | ones]."""
    import ml_dtypes

    w = np.asarray(weights, np.float32)
    s = np.asarray(adaptive_scale_factor, np.float32)
    wn = w / np.maximum(np.sqrt((w * w).sum(1, keepdims=True)), 1e-8)
    wn = wn * (s[:, None] * 10.0)
    wnT = wn.T  # [256, 80]
    wpack = np.empty((P, 3 * C), np.float32)
    wpack[:, 0:C] = wnT[:P]
    wpack[:, C : 2 * C] = wnT[P:]
    wpack[:, 2 * C : 3 * C] = 1.0
    return wpack.astype(ml_dtypes.bfloat16)


def prepare_in_maps(x, weights, adaptive_scale_factor, hw: int = HW):
    import ml_dtypes

    x = np.asarray(x, np.float32)
    wpack = prepare_wpack(weights, adaptive_scale_factor)
    return [
        {
            "x": np.ascontiguousarray(x[b].reshape(D, hw)).astype(ml_dtypes.bfloat16),
            "wpack": wpack,
        }
        for b in range(x.shape[0])
    ]


def kernel(x, weights, adaptive_scale_factor):
    from concourse.bass_utils import run_bass_kernel_spmd

    if "nc" not in _NC_CACHE:
        _NC_CACHE["nc"] = build_bass_kernel()
    nc = _NC_CACHE["nc"]

    in_maps = prepare_in_maps(x, weights, adaptive_scale_factor)
    res = run_bass_kernel_spmd(nc, in_maps, core_ids=list(range(N_CORES)))
    out = np.stack(
        [
            np.asarray(res.results[b]["out"], np.float32).reshape(C, 128, 128)
            for b in range(N_CORES)
        ]
    )
    return out


# revision 8
# speedup vs baseline: 1.0188x; 1.0188x over previous
"""CosHead kernel for Trainium2 (8 NeuronCores, data-parallel over batch).

Computes out[b,c,h,w] = 10 * scale[c] * cos_sim(x[b,:,h,w], weights[c,:])
 = (x[b,:,hw] . wn_scaled[c,:]) / ||x[b,:,hw]||
where wn_scaled[c,:] = weights[c,:] / ||weights[c,:]|| * scale[c] * 10.

v4 design (v1 f32: 86.8us, v2: 83.6us, v3: 69.0us):
  - bf16 end-to-end: x bf16 (8.4MB/core read), out bf16 (2.6MB/core
    write), upcast on host -> DMA floor ~31us/core at ~358GB/s share.
  - host-side weight prep uploaded as [128, 240] bf16 [wnT0|wnT1|ones].
  - norm inversion: ONE ACT pass via Abs_reciprocal_sqrt (direct
    InstActivation emission; dedicated TRN2 table, range 40000 >> norm^2
    ~450; measured rel err 4e-3 vs the 2e-2 budget).
  - v3 lesson (HAM log): PE oscillated k=8/8 <-> k=4/8 (27.5us at half
    duty, MM avg 456ns vs 215 warm) because in-order PE waited on
    same-group squares/psum consumers every 1024-col group. v4 runs a
    3-stage software pipeline at 512-col group granularity so every
    engine only touches results >= 1 group old:
      PE:  [gemm(g) x2MM, norm(g-1) x2MM]   (norm reads g-1's x^2)
      ACT: [sq_a(tile), rsqrt(g-2)]
      DVE: [sq_v(tile), mul(g-2)]
      GpSimd: [sq_g(tile), store-trigger(batch of 4 groups)]
    psum rings: pg (gemm) bufs=4, pn (norm) bufs=3 -> 7 of 8 banks; the
    deep pg ring lets PE run 2-3 groups ahead of the DVE muls.
  - squares per 1024-col load-tile, split by measured rates: DVE 832
    cols (bf16 2x_1P ~0.5ns/col), ACT 704 (~0.8ns/col), GpSimd 512
    (bf16 Q7 upconvert makes it slow: 1.76ns/col).
  - loads on sync HWDGE queue; stores batched [80, 2048] on gpsimd
    software-DGE, triggers one pipeline stage late so they never block.
"""

import os
import sys

import numpy as np

for _p in ("/opt/trn_rl_repo",):
    if os.path.isdir(_p) and _p not in sys.path:
        sys.path.append(_p)

B, D, C = 8, 256, 80
HW = 128 * 128
LTILE = 1024   # DMA load tile (hw cols)
G = 512        # pipeline group (hw cols) == one psum bank
P = 128        # SBUF partitions / d-chunk size
N_CORES = 8
STB = 4        # groups per output store batch

# squares: column split of each [128, 2*LTILE] block across engines.
# ACT deliberately takes none: rsqrt is on the critical pn->inv->mul
# chain, and keeping ACT slack (v5: 23us busy) measured faster than a
# fully balanced split (v6: ACT 35us busy, +2.3us total).
SQ_V = 1408  # DVE, bf16 2x_1P (~0.5ns/col)
SQ_A = 0     # ACT
# remainder (2*LTILE - SQ_V - SQ_A = 640) on GpSimd (bf16 1.76ns/col)

_NC_CACHE = {}


def _act_direct(nc, out, in_, func, scale=1.0):
    """Emit InstActivation directly (same lowering as nc.scalar.activation)
    for funcs the wrapper refuses (Abs_reciprocal_sqrt is table-backed on
    TRN2 but gated behind the Rsqrt accuracy warning)."""
    from concourse import mybir

    eng = nc.scalar
    bias = eng.bass.const_aps.scalar_like(0.0, in_)
    ins = [eng.lower_ap(in_), eng.lower_ap(bias)]
    ins.append(mybir.ImmediateValue(dtype=mybir.dt.float32, value=scale))
    ins.append(mybir.ImmediateValue(dtype=mybir.dt.float32, value=0.0))
    return eng.add_instruction(
        mybir.InstActivation(
            name=eng.bass.get_next_instruction_name(),
            func=func,
            ins=ins,
            outs=[eng.lower_ap(out)],
        )
    )


def build_bass_kernel(hw: int = HW, use_rsqrt: bool = True):
    """Build the single-core Bass program (SPMD: all cores run this)."""
    import concourse.bass as bass
    import concourse.tile as tile
    from concourse import bacc, mybir

    f32 = mybir.dt.float32
    bf16 = mybir.dt.bfloat16

    ng = hw // G          # pipeline groups
    gpt = LTILE // G      # groups per load tile

    nc = bacc.Bacc("TRN2", target_bir_lowering=False, debug=False)
    x_d = nc.declare_dram_parameter("x", [D, hw], bf16, isOutput=False)
    w_d = nc.declare_dram_parameter("wpack", [P, 3 * C], bf16, isOutput=False)
    out_d = nc.declare_dram_parameter("out", [C, hw], bf16, isOutput=True)

    with tile.TileContext(nc) as tc:
        with (
            tc.tile_pool(name="setup", bufs=1) as setup,
            tc.tile_pool(name="xp", bufs=5) as xp,
            tc.tile_pool(name="x2p", bufs=4) as x2p,
            tc.tile_pool(name="invp", bufs=3) as invp,
            tc.tile_pool(name="outp", bufs=3) as outp,
            tc.tile_pool(name="pg", bufs=4, space=bass.MemorySpace.PSUM) as pgp,
            tc.tile_pool(name="pn", bufs=3, space=bass.MemorySpace.PSUM) as pnp,
        ):
            wp = setup.tile([P, 3 * C], bf16)
            wnT0 = wp[:, 0:C]
            wnT1 = wp[:, C : 2 * C]
            ones = wp[:, 2 * C : 3 * C]

            # [256,hw] viewed as [128 partitions, 2 d-chunks, hw]
            x_src = x_d[:, :].rearrange("(c p) w -> p c w", c=2)
            wp_loaded = False

            xt = [None] * (hw // LTILE)   # load tiles
            x2t = [None] * (hw // LTILE)  # squared load tiles
            pgs = [None] * ng
            pns = [None] * ng
            invs = [None] * ng
            outs = [None] * (ng // STB)

            def xslice(g, chunk, x_list):
                lt, h = g // gpt, g % gpt
                base = chunk * LTILE + h * G
                return x_list[lt][:, base : base + G]

            for g in range(ng + 2):
                # -- load + squares, once per load tile --
                if g < ng and g % gpt == 0:
                    lt = g // gpt
                    lo = lt * LTILE
                    x_sb = xp.tile([P, 2 * LTILE], bf16)
                    nc.sync.dma_start(
                        out=x_sb[:].rearrange("p (c w) -> p c w", c=2),
                        in_=x_src[:, :, lo : lo + LTILE],
                    )
                    xt[lt] = x_sb
                    if not wp_loaded:
                        # weights load AFTER the first x tile: x is on the
                        # critical path, wpack only gates the first matmul
                        nc.sync.dma_start(out=wp, in_=w_d[:, :])
                        wp_loaded = True
                    x2_sb = x2p.tile([P, 2 * LTILE], bf16)
                    nc.vector.tensor_mul(
                        x2_sb[:, :SQ_V], x_sb[:, :SQ_V], x_sb[:, :SQ_V]
                    )
                    if SQ_A:
                        nc.scalar.square(
                            x2_sb[:, SQ_V : SQ_V + SQ_A],
                            x_sb[:, SQ_V : SQ_V + SQ_A],
                        )
                    nc.gpsimd.tensor_mul(
                        x2_sb[:, SQ_V + SQ_A :],
                        x_sb[:, SQ_V + SQ_A :],
                        x_sb[:, SQ_V + SQ_A :],
                    )
                    x2t[lt] = x2_sb

                # -- PE: norm(g-1) first (its deps are a stage older,
                # so the in-order PE is least likely to stall), then gemm(g)
                if 0 <= g - 1 < ng:
                    pn = pnp.tile([C, G], f32, tag="pn")
                    nc.tensor.matmul(
                        pn, ones, xslice(g - 1, 0, x2t), start=True, stop=False
                    )
                    nc.tensor.matmul(
                        pn, ones, xslice(g - 1, 1, x2t), start=False, stop=True
                    )
                    pns[g - 1] = pn
                if g < ng:
                    pg = pgp.tile([C, G], f32, tag="pg")
                    nc.tensor.matmul(
                        pg, wnT0, xslice(g, 0, xt), start=True, stop=False
                    )
                    nc.tensor.matmul(
                        pg, wnT1, xslice(g, 1, xt), start=False, stop=True
                    )
                    pgs[g] = pg

                # -- ACT: rsqrt(g-2); DVE: mul(g-2); GpSimd: store batch --
                k = g - 2
                if 0 <= k < ng:
                    inv = invp.tile([C, G], f32, tag="inv")
                    if use_rsqrt:
                        _act_direct(
                            nc, inv, pns[k],
                            mybir.ActivationFunctionType.Abs_reciprocal_sqrt,
                        )
                    else:
                        sq = invp.tile([C, G], f32, tag="sq")
                        nc.scalar.sqrt(sq, pns[k])
                        nc.vector.reciprocal_approx_fast(inv, sq)
                    invs[k] = inv
                    pns[k] = None

                    if k % STB == 0:
                        outs[k // STB] = outp.tile(
                            [C, STB * G], bf16, name=f"ob{k // STB}"
                        )
                    o_sb = outs[k // STB]
                    nc.vector.tensor_mul(
                        o_sb[:, (k % STB) * G : (k % STB + 1) * G],
                        pgs[k],
                        invs[k],
                    )
                    pgs[k] = None
                    invs[k] = None
                    if k % STB == STB - 1:
                        sb = k // STB
                        # last two batches drain on the scalar HWDGE queue:
                        # ACT is idle by then and hw-DGE drains faster than
                        # gpsimd software-DGE, shortening the tail
                        q = nc.scalar if sb >= (ng // STB) - 2 else nc.gpsimd
                        q.dma_start(
                            out=out_d[:, sb * STB * G : (sb + 1) * STB * G],
                            in_=o_sb,
                        )

    nc.compile()
    return nc


def prepare_wpack(weights, adaptive_scale_factor):
    """Host-side weight prep: [128, 240] bf16 = [wnT0 | wnT1 | ones]."""
    import ml_dtypes

    w = np.asarray(weights, np.float32)
    s = np.asarray(adaptive_scale_factor, np.float32)
    wn = w / np.maximum(np.sqrt((w * w).sum(1, keepdims=True)), 1e-8)
    wn = wn * (s[:, None] * 10.0)
    wnT = wn.T  # [256, 80]
    wpack = np.empty((P, 3 * C), np.float32)
    wpack[:, 0:C] = wnT[:P]
    wpack[:, C : 2 * C] = wnT[P:]
    wpack[:, 2 * C : 3 * C] = 1.0
    return wpack.astype(ml_dtypes.bfloat16)


def prepare_in_maps(x, weights, adaptive_scale_factor, hw: int = HW):
    import ml_dtypes

    x = np.asarray(x, np.float32)
    wpack = prepare_wpack(weights, adaptive_scale_factor)
    return [
        {
            "x": np.ascontiguousarray(x[b].reshape(D, hw)).astype(ml_dtypes.bfloat16),
            "wpack": wpack,
        }
        for b in range(x.shape[0])
    ]


def kernel(x, weights, adaptive_scale_factor):
    from concourse.bass_utils import run_bass_kernel_spmd

    if "nc" not in _NC_CACHE:
        _NC_CACHE["nc"] = build_bass_kernel()
    nc = _NC_CACHE["nc"]

    in_maps = prepare_in_maps(x, weights, adaptive_scale_factor)
    res = run_bass_kernel_spmd(nc, in_maps, core_ids=list(range(N_CORES)))
    out = np.stack(
        [
            np.asarray(res.results[b]["out"], np.float32).reshape(C, 128, 128)
            for b in range(N_CORES)
        ]
    )
    return out
